# revision 1
# baseline (speedup 1.0000x reference)
"""Trainium2 Bass kernel for DFNet-style dynamic-filter conv with K=2 MoE routing.

Math exploited:
  att = softmax([l0, l1]/T) with K=2  =>  att1 = sigmoid((l1-l0)/T), att0 = 1-att1
  agg_w(b) = att0*Wc + att1*We = Wc + s_b * (We - Wc),  s_b = att1
  same for the bias.  So per sample we need ONE scalar s_b, then generate the
  per-sample conv weight on-chip with one fused DVE op and run a regular
  stride-2 5x5 VALID conv as accumulated bf16 matmuls (fp32 PSUM).

Conv mapping (row-packed K=128): x arrives host-phase-split ([0:32] even
columns, [32:63] odd, col 63 zero pad) so every matmul reads contiguous
columns.  Per kernel column kw the 5*96=480 contraction items (kh x ci) are
packed into 4 matmuls (128/128/128/96 rows) using shadow partition blocks
holding x shifted by whole image rows (contiguous per-partition copies, done
as SBUF->SBUF DMAs off the compute engines):
  x_ext rows 96-127 = x[ci 0-31] shifted +1 image row
  t1    rows 0-63   = x[ci32-95]@+1 ; rows 64-127 = x[ci 0-63]@+2
  t2    rows 0-31   = x[ci64-95]@+2 ; rows 32-127 = x[ci 0-95]@+3
The matching packed weight layout is prepared on the host.

The emission is software-pipelined: x loads run 3 samples ahead, the
attention chain + shadow copies 2 ahead, and weight generation 1 ahead, so
the in-order PE stream never waits on the per-sample scalar chain.

Sharding: data-parallel over batch, 16 samples per core on 8 cores; expert
weights replicated.
"""

import numpy as np
import ml_dtypes
import concourse.bass as bass
import concourse.tile as tile
from concourse import bacc, mybir
from concourse.bass_utils import run_bass_kernel_spmd

B, CIN, H, W = 128, 96, 63, 63
WP = 64                 # phase-split padded width: [0:32] even cols, [32:63] odd
COUT, KS = 256, 5
HO = WO = 30
NCORES = 8
BL = B // NCORES        # 16 samples per core
NSLOT = 4               # packed matmuls per kernel row
NSG = KS * NSLOT        # 20 packed slots total
WFREE = NSG * COUT      # 5120 packed weight columns per partition
NPIX = HO * WO          # 900
NCH = NPIX // 2         # 450 output pixels per PSUM chunk (15 ho rows x 30 wo)
TEMP = 31.0

# packed-slot row segments: slot -> list of (p0, p1, ci0, kh); row p in
# [p0,p1) holds weight/input for channel ci0+(p-p0) at kernel row kh.
# Shadow tiles provide x shifted by whole image rows (contiguous copies).
SEGS = {
    0: [(0, 96, 0, 0), (96, 128, 0, 1)],
    1: [(0, 64, 32, 1), (64, 128, 0, 2)],
    2: [(0, 32, 64, 2), (32, 128, 0, 3)],
    3: [(0, 96, 0, 4)],
}

_f32 = mybir.dt.float32
_bf16 = mybir.dt.bfloat16
_AF = mybir.ActivationFunctionType
_OP = mybir.AluOpType


def _body(tc, xs, wc, dl, w1t, dvec, cbt, dbt, out, bl, reps=1):
    from contextlib import ExitStack

    nc = tc.nc
    with ExitStack() as ctx:
        consts = ctx.enter_context(tc.tile_pool(name="consts", bufs=1))
        xpool = ctx.enter_context(tc.tile_pool(name="xpool", bufs=5))
        tpool = ctx.enter_context(tc.tile_pool(name="tpool", bufs=3))
        wpool = ctx.enter_context(tc.tile_pool(name="wpool", bufs=2))
        spool = ctx.enter_context(tc.tile_pool(name="spool", bufs=4))
        opool = ctx.enter_context(tc.tile_pool(name="opool", bufs=4))
        pp = ctx.enter_context(tc.tile_pool(name="pp", bufs=7, space="PSUM"))
        pat = ctx.enter_context(tc.tile_pool(name="pat", bufs=1, space="PSUM"))

        wc_sb = consts.tile([128, WFREE], _bf16, name="wc_sb")
        nc.sync.dma_start(wc_sb[:], wc[:])
        dl_sb = consts.tile([128, WFREE], _bf16, name="dl_sb")
        nc.sync.dma_start(dl_sb[:], dl[:])
        w1t_sb = consts.tile([CIN, 24], _f32, name="w1t_sb")
        nc.sync.dma_start(w1t_sb[:], w1t[:])
        dvec_sb = consts.tile([24, 1], _f32, name="dvec_sb")
        nc.sync.dma_start(dvec_sb[:], dvec[:])
        cbt_sb = consts.tile([128, 2], _f32, name="cbt_sb")
        nc.sync.dma_start(cbt_sb[:], cbt[:])
        dbt_sb = consts.tile([128, 2], _f32, name="dbt_sb")
        nc.sync.dma_start(dbt_sb[:], dbt[:])
        ones_sb = consts.tile([1, 128], _f32, name="ones_sb")
        nc.vector.memset(ones_sb[:], 1.0)

        def sample_loop():
            xt, t1t, t2t = {}, {}, {}
            sbc, bia, wt = {}, {}, {}
            hidt, sst, sbps = {}, {}, {}

            def emit_load(b):
                xt[b] = xpool.tile([128, H, WP], _bf16, name="x_t", tag="x_t")
                nc.sync.dma_start(xt[b][0:CIN], xs[b])

            def emit_shadows(b):
                x_t = xt[b]
                # whole-row shifts; contiguous per partition -> 1 descriptor
                # per partition; partition moves are legal on the DMA path
                nc.gpsimd.dma_start(x_t[96:128, 0:62, :], x_t[0:32, 1:63, :])
                t1 = tpool.tile([128, H, WP], _bf16, name="t1", tag="t1")
                nc.gpsimd.dma_start(t1[0:64, 0:62, :], x_t[32:96, 1:63, :])
                nc.gpsimd.dma_start(t1[64:128, 0:61, :], x_t[0:64, 2:63, :])
                t2 = tpool.tile([128, H, WP], _bf16, name="t2", tag="t2")
                nc.gpsimd.dma_start(t2[0:32, 0:61, :], x_t[64:96, 2:63, :])
                nc.gpsimd.dma_start(t2[32:128, 0:60, :], x_t[0:96, 3:63, :])
                t1t[b], t2t[b] = t1, t2

            def emit_att1(b):
                pooled = spool.tile([CIN, 1], _f32, name="pooled", tag="pooled")
                nc.vector.tensor_reduce(
                    pooled[:], xt[b][0:CIN],
                    axis=mybir.AxisListType.XY, op=_OP.add,
                )
                h_ps = pat.tile([24, 1], _f32, name="h_ps", tag="aps")
                nc.tensor.matmul(h_ps[:], w1t_sb[:], pooled[:], start=True, stop=True)
                hid = spool.tile([24, 1], _f32, name="hid", tag="hid")
                nc.scalar.activation(hid[:], h_ps[:], _AF.Relu)
                hidt[b] = hid

            def emit_att2(b):
                d_ps = pat.tile([1, 1], _f32, name="d_ps", tag="aps")
                nc.tensor.matmul(d_ps[:], dvec_sb[:], hidt[b][:], start=True, stop=True)
                s_sb = spool.tile([1, 1], _f32, name="s_sb", tag="s_sb")
                nc.scalar.activation(s_sb[:], d_ps[:], _AF.Sigmoid)
                sst[b] = s_sb
                del hidt[b]

            def emit_att3(b):
                sb_ps = pat.tile([128, 1], _f32, name="sb_ps", tag="aps")
                nc.tensor.matmul(sb_ps[:], ones_sb[:], sst[b][:], start=True, stop=True)
                del sst[b]
                sbps[b] = sb_ps

            def emit_att4(b):
                s_bc = spool.tile([128, 1], _f32, name="s_bc", tag="s_bc")
                nc.scalar.activation(s_bc[:], sbps[b][:], _AF.Copy)
                del sbps[b]
                sbc[b] = s_bc
                bias_b = spool.tile([128, 2], _f32, name="bias_b", tag="bias_b")
                nc.vector.scalar_tensor_tensor(
                    bias_b[:], dbt_sb[:], s_bc[:], cbt_sb[:],
                    op0=_OP.mult, op1=_OP.add,
                )
                bia[b] = bias_b

            def emit_wgen(b):
                w_t = wpool.tile([128, WFREE], _bf16, name="w_t", tag="w_t")
                nc.vector.scalar_tensor_tensor(
                    w_t[:], dl_sb[:], sbc[b][:], wc_sb[:],
                    op0=_OP.mult, op1=_OP.add,
                )
                wt[b] = w_t

            def emit_conv(b, interleave=()):
                # interleave: callables fired between conv chunks so the next
                # samples' attention matmuls never stall the in-order PE.
                x_t, w_t, bias_b = xt[b], wt[b], bia[b]
                t1, t2 = t1t[b], t2t[b]
                il = list(interleave)
                for ct in range(2):
                    ob = opool.tile([128, NPIX], _bf16, name="ob", tag="ob")
                    for ch in range(2):
                        ps = pp.tile([128, NCH], _f32, name="ps", tag="ps")
                        rs = slice(30 * ch, 30 * ch + 29, 2)      # kh=0 base
                        rs4 = slice(4 + 30 * ch, 4 + 30 * ch + 29, 2)
                        for kw in range(KS):
                            # phase-split column block for this kw
                            p0 = kw // 2 if kw % 2 == 0 else 32 + (kw - 1) // 2
                            cs = slice(p0, p0 + 30)
                            for sl in range(NSLOT):
                                c0 = (kw * NSLOT + sl) * COUT + ct * 128
                                if sl == 0:
                                    rhs = x_t[:, rs, cs]
                                    lhsT = w_t[:, c0 : c0 + 128]
                                elif sl == 1:
                                    rhs = t1[:, rs, cs]
                                    lhsT = w_t[:, c0 : c0 + 128]
                                elif sl == 2:
                                    rhs = t2[:, rs, cs]
                                    lhsT = w_t[:, c0 : c0 + 128]
                                else:
                                    rhs = x_t[0:96, rs4, cs]
                                    lhsT = w_t[0:96, c0 : c0 + 128]
                                nc.tensor.matmul(
                                    ps[:], lhsT, rhs,
                                    start=(kw == 0 and sl == 0),
                                    stop=(kw == KS - 1 and sl == NSLOT - 1),
                                )
                        # PSUM->SBUF with fused bias add on the scalar engine
                        nc.scalar.activation(
                            ob[:, ch * NCH : (ch + 1) * NCH], ps[:],
                            _AF.Identity, bias=bias_b[:, ct : ct + 1],
                        )
                        if il:
                            il.pop(0)()
                    nc.sync.dma_start(out[b, ct * 128 : (ct + 1) * 128], ob[:])
                del xt[b], wt[b], bia[b], sbc[b], t1t[b], t2t[b]

            # software pipeline: loads 3 ahead, attention+shadows 2 ahead,
            # wgen 1 ahead
            def emit_att_all(b):
                emit_shadows(b)
                emit_att1(b)
                emit_att2(b)
                emit_att3(b)
                emit_att4(b)

            for b in range(min(3, bl)):
                emit_load(b)
            emit_att_all(0)
            emit_wgen(0)
            if bl > 1:
                emit_att_all(1)
            for b in range(bl):
                if b + 3 < bl:
                    emit_load(b + 3)
                if b + 1 < bl:
                    emit_wgen(b + 1)
                stages = []
                if b + 2 < bl:
                    stages = [lambda bb=b + 2: (emit_shadows(bb), emit_att1(bb)),
                              lambda bb=b + 2: emit_att2(bb),
                              lambda bb=b + 2: emit_att3(bb),
                              lambda bb=b + 2: emit_att4(bb)]
                emit_conv(b, interleave=stages)

        if reps == 1:
            sample_loop()
        else:
            with tc.For_i(0, reps, 1):
                sample_loop()


def build(bl=BL, reps=1):
    nc = bacc.Bacc("TRN2", target_bir_lowering=False, debug=False)
    xs = nc.dram_tensor("xs", [bl, CIN, H, WP], _bf16, kind="ExternalInput").ap()
    wc = nc.dram_tensor("wc", [128, WFREE], _bf16, kind="ExternalInput").ap()
    dl = nc.dram_tensor("dl", [128, WFREE], _bf16, kind="ExternalInput").ap()
    w1t = nc.dram_tensor("w1t", [CIN, 24], _f32, kind="ExternalInput").ap()
    dvec = nc.dram_tensor("dvec", [24, 1], _f32, kind="ExternalInput").ap()
    cbt = nc.dram_tensor("cbt", [128, 2], _f32, kind="ExternalInput").ap()
    dbt = nc.dram_tensor("dbt", [128, 2], _f32, kind="ExternalInput").ap()
    out = nc.dram_tensor("out", [bl, COUT, NPIX], _bf16, kind="ExternalOutput").ap()
    with tile.TileContext(nc) as tc:
        _body(tc, xs, wc, dl, w1t, dvec, cbt, dbt, out, bl, reps=reps)
    nc.compile()
    return nc


def _pack_w(wfull):
    """[COUT, CIN, KS, KS] -> packed lhsT [128, NSG*COUT] (kh-packed K=128)."""
    out = np.zeros((128, NSG * COUT), np.float32)
    for kw in range(KS):
        for sl in range(NSLOT):
            sg = kw * NSLOT + sl
            for (p0, p1, ci0, kh) in SEGS[sl]:
                n = p1 - p0
                out[p0:p1, sg * COUT : (sg + 1) * COUT] = \
                    wfull[:, ci0 : ci0 + n, kh, kw].T
    return out


def prep_inputs(x, common_weight, common_bias, expert_weight, expert_bias,
                att_w1, att_w2):
    """Host-side reshapes/packing of the small weight tensors (one-time)."""
    cw = np.asarray(common_weight, np.float32).reshape(COUT, CIN, KS, KS)
    ew = np.asarray(expert_weight, np.float32).reshape(COUT, CIN, KS, KS)
    wc_l = _pack_w(cw).astype(ml_dtypes.bfloat16)
    dl_l = _pack_w(ew - cw).astype(ml_dtypes.bfloat16)
    w1t = np.ascontiguousarray(np.asarray(att_w1, np.float32).T / float(H * W))
    dvec = np.ascontiguousarray(
        ((np.asarray(att_w2, np.float32)[1] - np.asarray(att_w2, np.float32)[0])
         / TEMP).reshape(24, 1))
    cb = np.asarray(common_bias, np.float32).reshape(COUT)
    eb = np.asarray(expert_bias, np.float32).reshape(COUT)
    cbt = np.ascontiguousarray(cb.reshape(2, 128).T)
    dbt = np.ascontiguousarray((eb - cb).reshape(2, 128).T)
    return wc_l, dl_l, w1t, dvec, cbt, dbt


def split_x(x):
    """Host phase-split + bf16: [*, CIN, H, W] -> [*, CIN, H, WP]."""
    x = np.asarray(x, np.float32)
    xp = np.zeros(x.shape[:-1] + (WP,), ml_dtypes.bfloat16)
    xp[..., 0:32] = x[..., 0::2]
    xp[..., 32:63] = x[..., 1::2]
    return xp


_NC_CACHE = None


def kernel(x, common_weight, common_bias, expert_weight, expert_bias,
           att_w1, att_w2):
    global _NC_CACHE
    if _NC_CACHE is None:
        _NC_CACHE = build()
    nc = _NC_CACHE
    x = np.asarray(x, np.float32)
    x_split = split_x(x)
    wc_l, dl_l, w1t, dvec, cbt, dbt = prep_inputs(
        x, common_weight, common_bias, expert_weight, expert_bias, att_w1, att_w2)
    shared = {"wc": wc_l, "dl": dl_l, "w1t": w1t, "dvec": dvec,
              "cbt": cbt, "dbt": dbt}
    in_maps = [{"xs": x_split[i * BL : (i + 1) * BL], **shared}
               for i in range(NCORES)]
    res = run_bass_kernel_spmd(nc, in_maps, list(range(NCORES))).results
    out = np.concatenate([np.asarray(res[i]["out"], np.float32)
                          for i in range(NCORES)], axis=0)
    return out.reshape(B, COUT, HO, WO)



# revision 2
# speedup vs baseline: 1.0453x; 1.0453x over previous
"""Trainium2 Bass kernel v2 for DFNet-style dynamic-filter conv (K=2 MoE).

Differences vs v1 baseline:
- Host row-phase-split layout [B, 2(row parity), CIN, 32, WP]: every shifted
  partition-packed tile loads DIRECTLY from HBM (no SBUF->SBUF shadow copies,
  no gpsimd dependency chain before the conv).
- Attention global-average-pool runs on the ACT engine via activation
  accum_out (2 reduces); DVE only does per-sample weight generation + bias.
- Deeper prefetch (x tiles 4 samples ahead), x loads on the sync HWDGE ring,
  output stores on the gpsimd SWDGE ring.
- Optional PACK19: consolidate the 5 per-kw kh=4 leftover matmuls into 4
  column-baked matmuls (19 instead of 20 contraction tiles per psum chunk).

Math (as v1): att1 = sigmoid((l1-l0)/T); agg_w(b) = Wc + att1 * (We - Wc).
"""

import os
import numpy as np
import ml_dtypes
import concourse.bass as bass
import concourse.tile as tile
from concourse import bacc, mybir
from concourse.bass_utils import run_bass_kernel_spmd

B, CIN, H, W = 128, 96, 63, 63
WP = 64                 # col phase split: [0:32] even cols, [32:63] odd, 63 pad
HP = 32                 # row phase split: plane0 = 32 even rows, plane1 = 31+pad
COUT, KS = 256, 5
HO = WO = 30
NCORES = 8
BL = B // NCORES
NPIX = HO * WO          # 900
NCH = NPIX // 2         # 450 per psum chunk (15 out rows x 30)
TEMP = 31.0
PACK19 = os.environ.get("K2_PACK19", "1") == "1"
ABL_NOXLOAD = os.environ.get("K2_NOXLOAD") == "1"
ABL_NOATT = os.environ.get("K2_NOATT") == "1"
ABL_NOWGEN = os.environ.get("K2_NOWGEN") == "1"
ABL_OUTSYNC = os.environ.get("K2_OUTSYNC", "1") == "1"
WGEN_MODE = os.environ.get("K2_WGEN", "dve")  # dve | gp | split
NGRP = 19 if PACK19 else 20
WFREE = NGRP * COUT

_f32 = mybir.dt.float32
_bf16 = mybir.dt.bfloat16
_AF = mybir.ActivationFunctionType
_OP = mybir.AluOpType

# kh -> (plane, baked row shift): kh0=(0,0) kh1=(1,0) kh2=(0,1) kh3=(1,1) kh4=(0,2)
# Tile partition layouts (ci ranges at given kh):
#  xt: p0:96 kh0 ci0-95      | p96:128 kh2 ci0-31
#  t1: p0:96 kh1 ci0-95      | p96:128 kh2 ci32-63
#  t2: p0:32 kh2 ci64-95     | p32:128 kh3 ci0-95
#  (pack20) slot3: kh4 ci0-95 read from xt[0:96] at row offset +2
#  (pack19) c0..c3: kh4 blocks column-baked per kw
SEGS3 = [
    [(0, 96, 0, 0), (96, 128, 0, 2)],
    [(0, 96, 0, 1), (96, 128, 32, 2)],
    [(0, 32, 64, 2), (32, 128, 0, 3)],
]
SEG_S3 = [(0, 96, 0, 4)]
# consolidated kh4 tiles: (p0, p1, ci0, kw)
CSEGS = [
    [(0, 96, 0, 0), (96, 128, 0, 1)],
    [(0, 64, 32, 1), (64, 128, 0, 2)],
    [(0, 32, 64, 2), (32, 128, 0, 3)],
    [(0, 96, 0, 4)],
]


def _colbase(kw):
    return kw // 2 if kw % 2 == 0 else 32 + (kw - 1) // 2


def _body(tc, xs, xc, wc, dl, w1t, dvec, cbt, dbt, out, bl, reps=1, opts=None):
    o = {"outsync": ABL_OUTSYNC, "wgen": WGEN_MODE, "noxload": ABL_NOXLOAD,
         "noatt": ABL_NOATT, "nowgen": ABL_NOWGEN}
    if opts:
        o.update(opts)
    from contextlib import ExitStack

    nc = tc.nc
    with ExitStack() as ctx:
        consts = ctx.enter_context(tc.tile_pool(name="consts", bufs=1))
        xtp = ctx.enter_context(tc.tile_pool(name="xtp", bufs=5))
        t1p = ctx.enter_context(tc.tile_pool(name="t1p", bufs=5))
        t2p = ctx.enter_context(tc.tile_pool(name="t2p", bufs=5))
        cpool = ctx.enter_context(tc.tile_pool(name="cpool", bufs=5))
        wpool = ctx.enter_context(tc.tile_pool(name="wpool", bufs=3))
        spool = ctx.enter_context(tc.tile_pool(name="spool", bufs=4))
        opool = ctx.enter_context(tc.tile_pool(name="opool", bufs=6))
        pp = ctx.enter_context(tc.tile_pool(name="pp", bufs=6, space="PSUM"))
        pat = ctx.enter_context(tc.tile_pool(name="pat", bufs=2, space="PSUM"))

        wc_sb = consts.tile([128, WFREE], _bf16, name="wc_sb")
        nc.sync.dma_start(wc_sb[:], wc[:])
        dl_sb = consts.tile([128, WFREE], _bf16, name="dl_sb")
        nc.sync.dma_start(dl_sb[:], dl[:])
        w1t_sb = consts.tile([CIN, 24], _f32, name="w1t_sb")
        nc.sync.dma_start(w1t_sb[:], w1t[:])
        dvec_sb = consts.tile([24, 1], _f32, name="dvec_sb")
        nc.sync.dma_start(dvec_sb[:], dvec[:])
        cbt_sb = consts.tile([128, 2], _f32, name="cbt_sb")
        nc.sync.dma_start(cbt_sb[:], cbt[:])
        dbt_sb = consts.tile([128, 2], _f32, name="dbt_sb")
        nc.sync.dma_start(dbt_sb[:], dbt[:])
        ones_sb = consts.tile([1, 128], _f32, name="ones_sb")
        nc.vector.memset(ones_sb[:], 1.0)
        scr = consts.tile([128, HP, WP], _bf16, name="scr")   # reduce dummy out
        sfix = consts.tile([128, 1], _f32, name="sfix")
        nc.vector.memset(sfix[:], 0.5)

        def sample_loop():
            xt, t1t, t2t, ctt = {}, {}, {}, {}
            accs, hidt, sst, sbps, sbc, bia, wt = {}, {}, {}, {}, {}, {}, {}

            def emit_load(b):
                if o["noxload"] and b > 0:
                    xt[b], t1t[b], t2t[b] = xt[0], t1t[0], t2t[0]
                    if PACK19:
                        ctt[b] = ctt[0]
                    return
                x_t = xtp.tile([128, HP, WP], _bf16, name="x_t", tag="x_t")
                t1 = t1p.tile([128, HP, WP], _bf16, name="t1", tag="t1")
                t2 = t2p.tile([128, HP, WP], _bf16, name="t2", tag="t2")
                nc.sync.dma_start(x_t[0:96, 0:32], xs[b, 0, 0:96, 0:32])
                nc.sync.dma_start(x_t[96:128, 0:31], xs[b, 0, 0:32, 1:32])
                nc.sync.dma_start(t1[0:96, 0:32], xs[b, 1, 0:96, 0:32])
                nc.sync.dma_start(t1[96:128, 0:31], xs[b, 0, 32:64, 1:32])
                nc.sync.dma_start(t2[0:32, 0:31], xs[b, 0, 64:96, 1:32])
                nc.sync.dma_start(t2[32:128, 0:31], xs[b, 1, 0:96, 1:32])
                xt[b], t1t[b], t2t[b] = x_t, t1, t2
                if PACK19:
                    cts = []
                    for j in range(4):
                        c_t = cpool.tile([128, 30, 30], _bf16,
                                         name=f"c{j}", tag=f"c{j}")
                        nc.sync.dma_start(c_t[:], xc[b, j])
                        cts.append(c_t)
                    ctt[b] = cts

            def att_a(b):
                if o["noatt"]:
                    return
                acc = spool.tile([CIN, 2], _f32, name="acc", tag="acc")
                nc.scalar.activation(scr[0:96], xt[b][0:96], _AF.Copy,
                                     accum_out=acc[:, 0:1])
                nc.scalar.activation(scr[0:96], t1t[b][0:96], _AF.Copy,
                                     accum_out=acc[:, 1:2])
                acc_s = spool.tile([CIN, 1], _f32, name="acc_s", tag="acc_s")
                nc.vector.scalar_tensor_tensor(
                    acc_s[:], acc[:, 0:1], 0.0, acc[:, 1:2],
                    op0=_OP.add, op1=_OP.add)
                accs[b] = acc_s

            def att_b(b):
                if o["noatt"]:
                    return
                h_ps = pat.tile([24, 1], _f32, name="h_ps", tag="aps")
                nc.tensor.matmul(h_ps[:], w1t_sb[:], accs[b][:], start=True, stop=True)
                del accs[b]
                hid = spool.tile([24, 1], _f32, name="hid", tag="hid")
                nc.scalar.activation(hid[:], h_ps[:], _AF.Relu)
                hidt[b] = hid

            def att_c(b):
                if o["noatt"]:
                    return
                d_ps = pat.tile([1, 1], _f32, name="d_ps", tag="aps")
                nc.tensor.matmul(d_ps[:], dvec_sb[:], hidt[b][:], start=True, stop=True)
                del hidt[b]
                s_sb = spool.tile([1, 1], _f32, name="s_sb", tag="s_sb")
                nc.scalar.activation(s_sb[:], d_ps[:], _AF.Sigmoid)
                sst[b] = s_sb

            def att_d(b):
                if o["noatt"]:
                    sbc[b] = sfix
                    bias_b = spool.tile([128, 2], _f32, name="bias_b", tag="bias_b")
                    nc.vector.scalar_tensor_tensor(
                        bias_b[:], dbt_sb[:], sfix[:], cbt_sb[:],
                        op0=_OP.mult, op1=_OP.add)
                    bia[b] = bias_b
                    return
                sb_ps = pat.tile([128, 1], _f32, name="sb_ps", tag="aps")
                nc.tensor.matmul(sb_ps[:], ones_sb[:], sst[b][:], start=True, stop=True)
                del sst[b]
                s_bc = spool.tile([128, 1], _f32, name="s_bc", tag="s_bc")
                nc.scalar.activation(s_bc[:], sb_ps[:], _AF.Copy)
                sbc[b] = s_bc
                bias_b = spool.tile([128, 2], _f32, name="bias_b", tag="bias_b")
                nc.vector.scalar_tensor_tensor(
                    bias_b[:], dbt_sb[:], s_bc[:], cbt_sb[:],
                    op0=_OP.mult, op1=_OP.add)
                bia[b] = bias_b

            def emit_wgen(b):
                if o["nowgen"]:
                    wt[b] = wc_sb
                    return
                w_t = wpool.tile([128, WFREE], _bf16, name="w_t", tag="w_t")
                hw = (WFREE // 2) // 2 * 2
                if o["wgen"] == "dve":
                    nc.vector.scalar_tensor_tensor(
                        w_t[:], dl_sb[:], sbc[b][:], wc_sb[:],
                        op0=_OP.mult, op1=_OP.add)
                elif o["wgen"] == "gp":
                    nc.gpsimd.scalar_tensor_tensor(
                        w_t[:], dl_sb[:], sbc[b][:], wc_sb[:],
                        op0=_OP.mult, op1=_OP.add)
                else:
                    nc.vector.scalar_tensor_tensor(
                        w_t[:, 0:hw], dl_sb[:, 0:hw], sbc[b][:], wc_sb[:, 0:hw],
                        op0=_OP.mult, op1=_OP.add)
                    nc.gpsimd.scalar_tensor_tensor(
                        w_t[:, hw:WFREE], dl_sb[:, hw:WFREE], sbc[b][:],
                        wc_sb[:, hw:WFREE], op0=_OP.mult, op1=_OP.add)
                wt[b] = w_t

            def emit_conv(b, interleave=()):
                x_t, t1, t2, w_t, bias_b = xt[b], t1t[b], t2t[b], wt[b], bia[b]
                il = list(interleave)
                for ct in range(2):
                    ob = opool.tile([128, NPIX], _bf16, name="ob", tag="ob")
                    for ch in range(2):
                        ps = pp.tile([128, NCH], _f32, name="ps", tag="ps")
                        rs = slice(15 * ch, 15 * ch + 15)
                        rs2 = slice(15 * ch + 2, 15 * ch + 17)
                        nsl = 3 if PACK19 else 4
                        g = 0
                        for kw in range(KS):
                            p0 = _colbase(kw)
                            cs = slice(p0, p0 + 30)
                            for sl in range(nsl):
                                c0 = g * COUT + ct * 128
                                g += 1
                                if sl == 0:
                                    rhs, lhsT = x_t[:, rs, cs], w_t[:, c0:c0 + 128]
                                elif sl == 1:
                                    rhs, lhsT = t1[:, rs, cs], w_t[:, c0:c0 + 128]
                                elif sl == 2:
                                    rhs, lhsT = t2[:, rs, cs], w_t[:, c0:c0 + 128]
                                else:
                                    rhs = x_t[0:96, rs2, cs]
                                    lhsT = w_t[0:96, c0:c0 + 128]
                                nc.tensor.matmul(
                                    ps[:], lhsT, rhs, start=(g == 1),
                                    stop=(not PACK19 and kw == KS - 1
                                          and sl == nsl - 1))
                        if PACK19:
                            for j in range(4):
                                c0 = g * COUT + ct * 128
                                g += 1
                                pc = 96 if j == 3 else 128
                                rhs = ctt[b][j][0:pc, rs, 0:30]
                                lhsT = w_t[0:pc, c0:c0 + 128]
                                nc.tensor.matmul(ps[:], lhsT, rhs,
                                                 start=False, stop=(j == 3))
                        else:
                            # close the accumulation group: reissue nothing;
                            # mark stop on the last emitted matmul instead
                            pass
                        nc.scalar.activation(
                            ob[:, ch * NCH:(ch + 1) * NCH], ps[:],
                            _AF.Identity, bias=bias_b[:, ct:ct + 1])
                        if il:
                            il.pop(0)()
                    if o["outsync"]:
                        nc.sync.dma_start(out[b, ct * 128:(ct + 1) * 128], ob[:])
                    else:
                        nc.gpsimd.dma_start(out[b, ct * 128:(ct + 1) * 128], ob[:])
                if not o["noxload"]:
                    del xt[b], t1t[b], t2t[b]
                    if PACK19:
                        del ctt[b]
                # (under ABL_NOXLOAD keep the shared sample-0 tiles)
                del wt[b], bia[b], sbc[b]

            def att_full(b):
                att_a(b); att_b(b); att_c(b); att_d(b)

            for b in range(min(4, bl)):
                emit_load(b)
            att_full(0)
            if bl > 1:
                att_full(1)
            emit_wgen(0)
            for b in range(bl):
                if b + 4 < bl:
                    emit_load(b + 4)
                if b + 1 < bl:
                    emit_wgen(b + 1)
                il = []
                if b + 2 < bl:
                    il = [lambda bb=b + 2: att_a(bb),
                          lambda bb=b + 2: att_b(bb),
                          lambda bb=b + 2: att_c(bb),
                          lambda bb=b + 2: att_d(bb)]
                emit_conv(b, interleave=il)

        if reps == 1:
            sample_loop()
        else:
            with tc.For_i(0, reps, 1):
                sample_loop()


def build(bl=BL, reps=1, opts=None):
    nc = bacc.Bacc("TRN2", target_bir_lowering=False, debug=False)
    xs = nc.dram_tensor("xs", [bl, 2, CIN, HP, WP], _bf16, kind="ExternalInput").ap()
    xc = nc.dram_tensor("xc", [bl, 4, 128, 30, 30], _bf16,
                        kind="ExternalInput").ap() if PACK19 else None
    wc = nc.dram_tensor("wc", [128, WFREE], _bf16, kind="ExternalInput").ap()
    dl = nc.dram_tensor("dl", [128, WFREE], _bf16, kind="ExternalInput").ap()
    w1t = nc.dram_tensor("w1t", [CIN, 24], _f32, kind="ExternalInput").ap()
    dvec = nc.dram_tensor("dvec", [24, 1], _f32, kind="ExternalInput").ap()
    cbt = nc.dram_tensor("cbt", [128, 2], _f32, kind="ExternalInput").ap()
    dbt = nc.dram_tensor("dbt", [128, 2], _f32, kind="ExternalInput").ap()
    out = nc.dram_tensor("out", [bl, COUT, NPIX], _bf16, kind="ExternalOutput").ap()
    with tile.TileContext(nc) as tc:
        _body(tc, xs, xc, wc, dl, w1t, dvec, cbt, dbt, out, bl, reps=reps, opts=opts)
    nc.compile()
    return nc


def _pack_w(wfull):
    """[COUT, CIN, KS, KS] -> packed lhsT [128, NGRP*COUT]."""
    out = np.zeros((128, WFREE), np.float32)
    g = 0
    for kw in range(KS):
        nsl = 3 if PACK19 else 4
        for sl in range(nsl):
            segs = SEGS3[sl] if sl < 3 else SEG_S3
            for (p0, p1, ci0, kh) in segs:
                out[p0:p1, g * COUT:(g + 1) * COUT] = \
                    wfull[:, ci0:ci0 + (p1 - p0), kh, kw].T
            g += 1
    if PACK19:
        for segs in CSEGS:
            for (p0, p1, ci0, kw) in segs:
                out[p0:p1, g * COUT:(g + 1) * COUT] = \
                    wfull[:, ci0:ci0 + (p1 - p0), 4, kw].T
            g += 1
    assert g == NGRP
    return out


def prep_inputs(x, common_weight, common_bias, expert_weight, expert_bias,
                att_w1, att_w2):
    cw = np.asarray(common_weight, np.float32).reshape(COUT, CIN, KS, KS)
    ew = np.asarray(expert_weight, np.float32).reshape(COUT, CIN, KS, KS)
    wc_l = _pack_w(cw).astype(ml_dtypes.bfloat16)
    dl_l = _pack_w(ew - cw).astype(ml_dtypes.bfloat16)
    w1t = np.ascontiguousarray(np.asarray(att_w1, np.float32).T / float(H * W))
    dvec = np.ascontiguousarray(
        ((np.asarray(att_w2, np.float32)[1] - np.asarray(att_w2, np.float32)[0])
         / TEMP).reshape(24, 1))
    cb = np.asarray(common_bias, np.float32).reshape(COUT)
    eb = np.asarray(expert_bias, np.float32).reshape(COUT)
    cbt = np.ascontiguousarray(cb.reshape(2, 128).T)
    dbt = np.ascontiguousarray((eb - cb).reshape(2, 128).T)
    return wc_l, dl_l, w1t, dvec, cbt, dbt


def split_x(x):
    """[*, CIN, H, W] -> [*, 2, CIN, HP, WP] bf16 (row+col phase split)."""
    x = np.asarray(x, np.float32)
    lead = x.shape[:-3]
    xp = np.zeros(lead + (2, CIN, HP, WP), ml_dtypes.bfloat16)
    xe = x[..., 0::2, :]                       # 32 even rows
    xo = x[..., 1::2, :]                       # 31 odd rows
    xp[..., 0, :, :, 0:32] = xe[..., 0::2]
    xp[..., 0, :, :, 32:63] = xe[..., 1::2]
    xp[..., 1, :, 0:31, 0:32] = xo[..., 0::2]
    xp[..., 1, :, 0:31, 32:63] = xo[..., 1::2]
    return xp


def pack_c(x):
    """[*, CIN, H, W] -> [*, 4, 128, 30, 30] bf16: kh=4 col-baked tiles."""
    x = np.asarray(x, np.float32)
    lead = x.shape[:-3]
    out = np.zeros(lead + (4, 128, 30, 30), ml_dtypes.bfloat16)
    for j, segs in enumerate(CSEGS):
        for (p0, p1, ci0, kw) in segs:
            blk = x[..., ci0:ci0 + (p1 - p0), 4::2, kw::2][..., :30, :30]
            out[..., j, p0:p1, :, :] = blk
    return out


_NC_CACHE = None


def kernel(x, common_weight, common_bias, expert_weight, expert_bias,
           att_w1, att_w2):
    global _NC_CACHE
    if _NC_CACHE is None:
        _NC_CACHE = build()
    nc = _NC_CACHE
    x_split = split_x(np.asarray(x, np.float32))
    x_c = pack_c(x) if PACK19 else None
    wc_l, dl_l, w1t, dvec, cbt, dbt = prep_inputs(
        x, common_weight, common_bias, expert_weight, expert_bias, att_w1, att_w2)
    shared = {"wc": wc_l, "dl": dl_l, "w1t": w1t, "dvec": dvec,
              "cbt": cbt, "dbt": dbt}
    in_maps = []
    for i in range(NCORES):
        m = {"xs": x_split[i * BL:(i + 1) * BL], **shared}
        if PACK19:
            m["xc"] = x_c[i * BL:(i + 1) * BL]
        in_maps.append(m)
    res = run_bass_kernel_spmd(nc, in_maps, list(range(NCORES))).results
    out = np.concatenate([np.asarray(res[i]["out"], np.float32)
                          for i in range(NCORES)], axis=0)
    return out.reshape(B, COUT, HO, WO)


# revision 3
# speedup vs baseline: 3.4044x; 3.2570x over previous
"""Trainium2 Bass kernel v2 for DFNet-style dynamic-filter conv (K=2 MoE).

Differences vs v1 baseline:
- Host row-phase-split layout [B, 2(row parity), CIN, 32, WP]: every shifted
  partition-packed tile loads DIRECTLY from HBM (no SBUF->SBUF shadow copies,
  no gpsimd dependency chain before the conv).
- Attention global-average-pool runs on the ACT engine via activation
  accum_out (2 reduces); DVE only does per-sample weight generation + bias.
- Deeper prefetch (x tiles 4 samples ahead), x loads on the sync HWDGE ring,
  output stores on the gpsimd SWDGE ring.
- Optional PACK19: consolidate the 5 per-kw kh=4 leftover matmuls into 4
  column-baked matmuls (19 instead of 20 contraction tiles per psum chunk).

Math (as v1): att1 = sigmoid((l1-l0)/T); agg_w(b) = Wc + att1 * (We - Wc).
"""

import os
import numpy as np
import ml_dtypes
import concourse.bass as bass
import concourse.tile as tile
from concourse import bacc, mybir
from concourse.bass_utils import run_bass_kernel_spmd

B, CIN, H, W = 128, 96, 63, 63
WP = 64                 # col phase split: [0:32] even cols, [32:63] odd, 63 pad
HP = 32                 # row phase split: plane0 = 32 even rows, plane1 = 31+pad
COUT, KS = 256, 5
HO = WO = 30
NCORES = 8
BL = B // NCORES
NPIX = HO * WO          # 900
NCH = NPIX // 2         # 450 per psum chunk (15 out rows x 30)
TEMP = 31.0
PACK19 = os.environ.get("K2_PACK19", "1") == "1"
ABL_NOXLOAD = os.environ.get("K2_NOXLOAD") == "1"
ABL_NOATT = os.environ.get("K2_NOATT") == "1"
ABL_NOWGEN = os.environ.get("K2_NOWGEN") == "1"
ABL_OUTSYNC = os.environ.get("K2_OUTSYNC", "1") == "1"
WGEN_MODE = os.environ.get("K2_WGEN", "dve")  # dve | gp | split
NGRP = 19 if PACK19 else 20
WFREE = NGRP * COUT

_f32 = mybir.dt.float32
_bf16 = mybir.dt.bfloat16
_AF = mybir.ActivationFunctionType
_OP = mybir.AluOpType

# kh -> (plane, baked row shift): kh0=(0,0) kh1=(1,0) kh2=(0,1) kh3=(1,1) kh4=(0,2)
# Tile partition layouts (ci ranges at given kh):
#  xt: p0:96 kh0 ci0-95      | p96:128 kh2 ci0-31
#  t1: p0:96 kh1 ci0-95      | p96:128 kh2 ci32-63
#  t2: p0:32 kh2 ci64-95     | p32:128 kh3 ci0-95
#  (pack20) slot3: kh4 ci0-95 read from xt[0:96] at row offset +2
#  (pack19) c0..c3: kh4 blocks column-baked per kw
SEGS3 = [
    [(0, 96, 0, 0), (96, 128, 0, 2)],
    [(0, 96, 0, 1), (96, 128, 32, 2)],
    [(0, 32, 64, 2), (32, 128, 0, 3)],
]
SEG_S3 = [(0, 96, 0, 4)]
# consolidated kh4 tiles: (p0, p1, ci0, kw)
CSEGS = [
    [(0, 96, 0, 0), (96, 128, 0, 1)],
    [(0, 64, 32, 1), (64, 128, 0, 2)],
    [(0, 32, 64, 2), (32, 128, 0, 3)],
    [(0, 96, 0, 4)],
]


def _colbase(kw):
    return kw // 2 if kw % 2 == 0 else 32 + (kw - 1) // 2


def _body(tc, xs, xc, wc, dl, w1t, dvec, cbt, dbt, out, bl, reps=1, opts=None):
    o = {"outsync": ABL_OUTSYNC, "wgen": WGEN_MODE, "noxload": ABL_NOXLOAD,
         "noatt": ABL_NOATT, "nowgen": ABL_NOWGEN}
    if opts:
        o.update(opts)
    from contextlib import ExitStack

    nc = tc.nc
    with ExitStack() as ctx:
        consts = ctx.enter_context(tc.tile_pool(name="consts", bufs=1))
        xtp = ctx.enter_context(tc.tile_pool(name="xtp", bufs=5))
        t1p = ctx.enter_context(tc.tile_pool(name="t1p", bufs=5))
        t2p = ctx.enter_context(tc.tile_pool(name="t2p", bufs=5))
        cpool = ctx.enter_context(tc.tile_pool(name="cpool", bufs=5))
        wpool = ctx.enter_context(tc.tile_pool(name="wpool", bufs=3))
        spool = ctx.enter_context(tc.tile_pool(name="spool", bufs=4))
        opool = ctx.enter_context(tc.tile_pool(name="opool", bufs=6))
        pp = ctx.enter_context(tc.tile_pool(name="pp", bufs=6, space="PSUM"))
        pat = ctx.enter_context(tc.tile_pool(name="pat", bufs=2, space="PSUM"))

        wc_sb = consts.tile([128, WFREE], _bf16, name="wc_sb")
        nc.sync.dma_start(wc_sb[:], wc[:])
        dl_sb = consts.tile([128, WFREE], _bf16, name="dl_sb")
        nc.sync.dma_start(dl_sb[:], dl[:])
        w1t_sb = consts.tile([CIN, 24], _f32, name="w1t_sb")
        nc.sync.dma_start(w1t_sb[:], w1t[:])
        dvec_sb = consts.tile([24, 1], _f32, name="dvec_sb")
        nc.sync.dma_start(dvec_sb[:], dvec[:])
        cbt_sb = consts.tile([128, 2], _f32, name="cbt_sb")
        nc.sync.dma_start(cbt_sb[:], cbt[:])
        dbt_sb = consts.tile([128, 2], _f32, name="dbt_sb")
        nc.sync.dma_start(dbt_sb[:], dbt[:])
        ones_sb = consts.tile([1, 128], _f32, name="ones_sb")
        nc.vector.memset(ones_sb[:], 1.0)
        scr = consts.tile([128, HP, WP], _bf16, name="scr")   # reduce dummy out
        sfix = consts.tile([128, 1], _f32, name="sfix")
        nc.vector.memset(sfix[:], 0.5)

        def sample_loop():
            xt, t1t, t2t, ctt = {}, {}, {}, {}
            accs, hidt, sst, sbps, sbc, bia, wt = {}, {}, {}, {}, {}, {}, {}

            def emit_load(b):
                if o["noxload"] and b > 0:
                    xt[b], t1t[b], t2t[b] = xt[0], t1t[0], t2t[0]
                    if PACK19:
                        ctt[b] = ctt[0]
                    return
                x_t = xtp.tile([128, HP, WP], _bf16, name="x_t", tag="x_t")
                t1 = t1p.tile([128, HP, WP], _bf16, name="t1", tag="t1")
                t2 = t2p.tile([128, HP, WP], _bf16, name="t2", tag="t2")
                nc.sync.dma_start(x_t[0:96, 0:32], xs[b, 0, 0:96, 0:32])
                nc.sync.dma_start(x_t[96:128, 0:31], xs[b, 0, 0:32, 1:32])
                nc.sync.dma_start(t1[0:96, 0:32], xs[b, 1, 0:96, 0:32])
                nc.sync.dma_start(t1[96:128, 0:31], xs[b, 0, 32:64, 1:32])
                nc.scalar.dma_start(t2[0:32, 0:31], xs[b, 0, 64:96, 1:32])
                nc.scalar.dma_start(t2[32:128, 0:31], xs[b, 1, 0:96, 1:32])
                xt[b], t1t[b], t2t[b] = x_t, t1, t2
                if PACK19:
                    cts = []
                    for j in range(4):
                        c_t = cpool.tile([128, 30, 30], _bf16,
                                         name=f"c{j}", tag=f"c{j}")
                        nc.scalar.dma_start(c_t[:], xc[b, j])
                        cts.append(c_t)
                    ctt[b] = cts

            def att_a(b):
                if o["noatt"]:
                    return
                acc = spool.tile([CIN, 2], _f32, name="acc", tag="acc")
                nc.scalar.activation(scr[0:96], xt[b][0:96], _AF.Copy,
                                     accum_out=acc[:, 0:1])
                nc.scalar.activation(scr[0:96], t1t[b][0:96], _AF.Copy,
                                     accum_out=acc[:, 1:2])
                acc_s = spool.tile([CIN, 1], _f32, name="acc_s", tag="acc_s")
                nc.vector.scalar_tensor_tensor(
                    acc_s[:], acc[:, 0:1], 0.0, acc[:, 1:2],
                    op0=_OP.add, op1=_OP.add)
                accs[b] = acc_s

            def att_b(b):
                if o["noatt"]:
                    return
                h_ps = pat.tile([24, 1], _f32, name="h_ps", tag="aps")
                nc.tensor.matmul(h_ps[:], w1t_sb[:], accs[b][:], start=True, stop=True)
                del accs[b]
                hid = spool.tile([24, 1], _f32, name="hid", tag="hid")
                nc.scalar.activation(hid[:], h_ps[:], _AF.Relu)
                hidt[b] = hid

            def att_c(b):
                if o["noatt"]:
                    return
                d_ps = pat.tile([1, 1], _f32, name="d_ps", tag="aps")
                nc.tensor.matmul(d_ps[:], dvec_sb[:], hidt[b][:], start=True, stop=True)
                del hidt[b]
                s_sb = spool.tile([1, 1], _f32, name="s_sb", tag="s_sb")
                nc.scalar.activation(s_sb[:], d_ps[:], _AF.Sigmoid)
                sst[b] = s_sb

            def att_d(b):
                if o["noatt"]:
                    sbc[b] = sfix
                    bias_b = spool.tile([128, 2], _f32, name="bias_b", tag="bias_b")
                    nc.vector.scalar_tensor_tensor(
                        bias_b[:], dbt_sb[:], sfix[:], cbt_sb[:],
                        op0=_OP.mult, op1=_OP.add)
                    bia[b] = bias_b
                    return
                sb_ps = pat.tile([128, 1], _f32, name="sb_ps", tag="aps")
                nc.tensor.matmul(sb_ps[:], ones_sb[:], sst[b][:], start=True, stop=True)
                del sst[b]
                s_bc = spool.tile([128, 1], _f32, name="s_bc", tag="s_bc")
                nc.scalar.activation(s_bc[:], sb_ps[:], _AF.Copy)
                sbc[b] = s_bc
                bias_b = spool.tile([128, 2], _f32, name="bias_b", tag="bias_b")
                nc.vector.scalar_tensor_tensor(
                    bias_b[:], dbt_sb[:], s_bc[:], cbt_sb[:],
                    op0=_OP.mult, op1=_OP.add)
                bia[b] = bias_b

            def emit_wgen(b):
                if o["nowgen"]:
                    wt[b] = wc_sb
                    return
                w_t = wpool.tile([128, WFREE], _bf16, name="w_t", tag="w_t")
                hw = (WFREE // 2) // 2 * 2
                if o["wgen"] == "dve":
                    nc.vector.scalar_tensor_tensor(
                        w_t[:], dl_sb[:], sbc[b][:], wc_sb[:],
                        op0=_OP.mult, op1=_OP.add)
                elif o["wgen"] == "gp":
                    nc.gpsimd.scalar_tensor_tensor(
                        w_t[:], dl_sb[:], sbc[b][:], wc_sb[:],
                        op0=_OP.mult, op1=_OP.add)
                else:
                    nc.vector.scalar_tensor_tensor(
                        w_t[:, 0:hw], dl_sb[:, 0:hw], sbc[b][:], wc_sb[:, 0:hw],
                        op0=_OP.mult, op1=_OP.add)
                    nc.gpsimd.scalar_tensor_tensor(
                        w_t[:, hw:WFREE], dl_sb[:, hw:WFREE], sbc[b][:],
                        wc_sb[:, hw:WFREE], op0=_OP.mult, op1=_OP.add)
                wt[b] = w_t

            def emit_conv(b, interleave=()):
                x_t, t1, t2, w_t, bias_b = xt[b], t1t[b], t2t[b], wt[b], bia[b]
                il = list(interleave)
                for ct in range(2):
                    ob = opool.tile([128, NPIX], _bf16, name="ob", tag="ob")
                    for ch in range(2):
                        ps = pp.tile([128, NCH], _f32, name="ps", tag="ps")
                        rs = slice(15 * ch, 15 * ch + 15)
                        rs2 = slice(15 * ch + 2, 15 * ch + 17)
                        nsl = 3 if PACK19 else 4
                        g = 0
                        for kw in range(KS):
                            p0 = _colbase(kw)
                            cs = slice(p0, p0 + 30)
                            for sl in range(nsl):
                                c0 = g * COUT + ct * 128
                                g += 1
                                if sl == 0:
                                    rhs, lhsT = x_t[:, rs, cs], w_t[:, c0:c0 + 128]
                                elif sl == 1:
                                    rhs, lhsT = t1[:, rs, cs], w_t[:, c0:c0 + 128]
                                elif sl == 2:
                                    rhs, lhsT = t2[:, rs, cs], w_t[:, c0:c0 + 128]
                                else:
                                    rhs = x_t[0:96, rs2, cs]
                                    lhsT = w_t[0:96, c0:c0 + 128]
                                nc.tensor.matmul(
                                    ps[:], lhsT, rhs, start=(g == 1),
                                    stop=(not PACK19 and kw == KS - 1
                                          and sl == nsl - 1))
                        if PACK19:
                            for j in range(4):
                                c0 = g * COUT + ct * 128
                                g += 1
                                pc = 96 if j == 3 else 128
                                rhs = ctt[b][j][0:pc, rs, 0:30]
                                lhsT = w_t[0:pc, c0:c0 + 128]
                                nc.tensor.matmul(ps[:], lhsT, rhs,
                                                 start=False, stop=(j == 3))
                        else:
                            # close the accumulation group: reissue nothing;
                            # mark stop on the last emitted matmul instead
                            pass
                        nc.scalar.activation(
                            ob[:, ch * NCH:(ch + 1) * NCH], ps[:],
                            _AF.Identity, bias=bias_b[:, ct:ct + 1])
                        if il:
                            il.pop(0)()
                    if o["outsync"]:
                        nc.sync.dma_start(out[b, ct * 128:(ct + 1) * 128], ob[:])
                    else:
                        nc.gpsimd.dma_start(out[b, ct * 128:(ct + 1) * 128], ob[:])
                if not o["noxload"]:
                    del xt[b], t1t[b], t2t[b]
                    if PACK19:
                        del ctt[b]
                # (under ABL_NOXLOAD keep the shared sample-0 tiles)
                del wt[b], bia[b], sbc[b]

            def att_full(b):
                att_a(b); att_b(b); att_c(b); att_d(b)

            for b in range(min(4, bl)):
                emit_load(b)
            att_full(0)
            if bl > 1:
                att_full(1)
            emit_wgen(0)
            for b in range(bl):
                if b + 4 < bl:
                    emit_load(b + 4)
                if b + 1 < bl:
                    emit_wgen(b + 1)
                il = []
                if b + 2 < bl:
                    il = [lambda bb=b + 2: att_a(bb),
                          lambda bb=b + 2: att_b(bb),
                          lambda bb=b + 2: att_c(bb),
                          lambda bb=b + 2: att_d(bb)]
                emit_conv(b, interleave=il)

        if reps == 1:
            sample_loop()
        else:
            with tc.For_i(0, reps, 1):
                sample_loop()


def build(bl=BL, reps=1, opts=None):
    nc = bacc.Bacc("TRN2", target_bir_lowering=False, debug=False)
    xs = nc.dram_tensor("xs", [bl, 2, CIN, HP, WP], _bf16, kind="ExternalInput").ap()
    xc = nc.dram_tensor("xc", [bl, 4, 128, 30, 30], _bf16,
                        kind="ExternalInput").ap() if PACK19 else None
    wc = nc.dram_tensor("wc", [128, WFREE], _bf16, kind="ExternalInput").ap()
    dl = nc.dram_tensor("dl", [128, WFREE], _bf16, kind="ExternalInput").ap()
    w1t = nc.dram_tensor("w1t", [CIN, 24], _f32, kind="ExternalInput").ap()
    dvec = nc.dram_tensor("dvec", [24, 1], _f32, kind="ExternalInput").ap()
    cbt = nc.dram_tensor("cbt", [128, 2], _f32, kind="ExternalInput").ap()
    dbt = nc.dram_tensor("dbt", [128, 2], _f32, kind="ExternalInput").ap()
    out = nc.dram_tensor("out", [bl, COUT, NPIX], _bf16, kind="ExternalOutput").ap()
    with tile.TileContext(nc) as tc:
        _body(tc, xs, xc, wc, dl, w1t, dvec, cbt, dbt, out, bl, reps=reps, opts=opts)
    nc.compile()
    return nc


def _pack_w(wfull):
    """[COUT, CIN, KS, KS] -> packed lhsT [128, NGRP*COUT]."""
    out = np.zeros((128, WFREE), np.float32)
    g = 0
    for kw in range(KS):
        nsl = 3 if PACK19 else 4
        for sl in range(nsl):
            segs = SEGS3[sl] if sl < 3 else SEG_S3
            for (p0, p1, ci0, kh) in segs:
                out[p0:p1, g * COUT:(g + 1) * COUT] = \
                    wfull[:, ci0:ci0 + (p1 - p0), kh, kw].T
            g += 1
    if PACK19:
        for segs in CSEGS:
            for (p0, p1, ci0, kw) in segs:
                out[p0:p1, g * COUT:(g + 1) * COUT] = \
                    wfull[:, ci0:ci0 + (p1 - p0), 4, kw].T
            g += 1
    assert g == NGRP
    return out


def prep_inputs(x, common_weight, common_bias, expert_weight, expert_bias,
                att_w1, att_w2):
    cw = np.asarray(common_weight, np.float32).reshape(COUT, CIN, KS, KS)
    ew = np.asarray(expert_weight, np.float32).reshape(COUT, CIN, KS, KS)
    wc_l = _pack_w(cw).astype(ml_dtypes.bfloat16)
    dl_l = _pack_w(ew - cw).astype(ml_dtypes.bfloat16)
    w1t = np.ascontiguousarray(np.asarray(att_w1, np.float32).T / float(H * W))
    dvec = np.ascontiguousarray(
        ((np.asarray(att_w2, np.float32)[1] - np.asarray(att_w2, np.float32)[0])
         / TEMP).reshape(24, 1))
    cb = np.asarray(common_bias, np.float32).reshape(COUT)
    eb = np.asarray(expert_bias, np.float32).reshape(COUT)
    cbt = np.ascontiguousarray(cb.reshape(2, 128).T)
    dbt = np.ascontiguousarray((eb - cb).reshape(2, 128).T)
    return wc_l, dl_l, w1t, dvec, cbt, dbt


def split_x(x):
    """[*, CIN, H, W] -> [*, 2, CIN, HP, WP] bf16 (row+col phase split)."""
    x = np.asarray(x, np.float32)
    lead = x.shape[:-3]
    xp = np.zeros(lead + (2, CIN, HP, WP), ml_dtypes.bfloat16)
    xe = x[..., 0::2, :]                       # 32 even rows
    xo = x[..., 1::2, :]                       # 31 odd rows
    xp[..., 0, :, :, 0:32] = xe[..., 0::2]
    xp[..., 0, :, :, 32:63] = xe[..., 1::2]
    xp[..., 1, :, 0:31, 0:32] = xo[..., 0::2]
    xp[..., 1, :, 0:31, 32:63] = xo[..., 1::2]
    return xp


def pack_c(x):
    """[*, CIN, H, W] -> [*, 4, 128, 30, 30] bf16: kh=4 col-baked tiles."""
    x = np.asarray(x, np.float32)
    lead = x.shape[:-3]
    out = np.zeros(lead + (4, 128, 30, 30), ml_dtypes.bfloat16)
    for j, segs in enumerate(CSEGS):
        for (p0, p1, ci0, kw) in segs:
            blk = x[..., ci0:ci0 + (p1 - p0), 4::2, kw::2][..., :30, :30]
            out[..., j, p0:p1, :, :] = blk
    return out


_NC_CACHE = None


def kernel(x, common_weight, common_bias, expert_weight, expert_bias,
           att_w1, att_w2):
    global _NC_CACHE
    if _NC_CACHE is None:
        _NC_CACHE = build()
    nc = _NC_CACHE
    x_split = split_x(np.asarray(x, np.float32))
    x_c = pack_c(x) if PACK19 else None
    wc_l, dl_l, w1t, dvec, cbt, dbt = prep_inputs(
        x, common_weight, common_bias, expert_weight, expert_bias, att_w1, att_w2)
    shared = {"wc": wc_l, "dl": dl_l, "w1t": w1t, "dvec": dvec,
              "cbt": cbt, "dbt": dbt}
    in_maps = []
    for i in range(NCORES):
        m = {"xs": x_split[i * BL:(i + 1) * BL], **shared}
        if PACK19:
            m["xc"] = x_c[i * BL:(i + 1) * BL]
        in_maps.append(m)
    res = run_bass_kernel_spmd(nc, in_maps, list(range(NCORES))).results
    out = np.concatenate([np.asarray(res[i]["out"], np.float32)
                          for i in range(NCORES)], axis=0)
    return out.reshape(B, COUT, HO, WO)
